# revision 4
# baseline (speedup 1.0000x reference)
"""Trainium2 Bass kernel for PVT-style spatial-reduction attention.

Problem (per batch element b of 8, one NeuronCore each — pure data parallel):
  q  = x @ Wq + bq                                  [16384, 64]
  xs = conv8x8s8(x.reshape(128,128,64), Wsr) + bsr  [256, 64]
  xs = LayerNorm(xs) * gamma + beta
  k  = xs @ Wk + bk ; v = xs @ Wv + bv              [256, 64]
  A  = softmax(q @ k.T / 8) ; o = A @ v             [16384, 64]
  out = o @ Wp + bp

Algebraic restructuring used on-device (all layouts channel-on-partition):
  - No Q tensor at all:  S^T[m, n] = sum_c kq[c, m] xT[c, n]
      with kq = Wq^T @ k^T  (i.e. Q-projection folded into K).
  - bq handled via d[m] = exp((k @ bq) / 8), folded diagonally into V:
      softmax(S + bqk 1^T) @ V == softmax-unnorm(S) @ diag(d) V / rowsum.
  - gamma/beta folded into Wk/Wv (+ bias terms), Wp+bp folded into V:
      vp = (xs_n @ Wvg + bvf) @ Wp + bp ; vp_aug = diag(d) [vp | 1].
  - Attention output computed directly in token layout with E = exp(S^T/8)
    as the matmul stationary operand:  y_un[n, :] = E^T-slices.T @ vp_aug,
    last column = softmax denominator; one reciprocal+multiply finishes.
"""

import os
import sys

import numpy as np

for _p in ("/root/.axon_site", "/root/.axon_site/_ro/trn_rl_repo",
           "/root/.axon_site/_ro/pypackages", "/opt/trn_rl_repo"):
    if os.path.isdir(_p) and _p not in sys.path:
        sys.path.append(_p)

import concourse.bass as bass  # noqa: E402
import concourse.mybir as mybir  # noqa: E402
import concourse.tile as tile  # noqa: E402
from concourse import bacc  # noqa: E402
from concourse.bass_utils import run_bass_kernel_spmd  # noqa: E402
from concourse.masks import make_identity  # noqa: E402

F32 = mybir.dt.float32
F32R = mybir.dt.float32r
BF16 = mybir.dt.bfloat16
AF = mybir.ActivationFunctionType

N_CORES = 8
N = 16384          # tokens per core (H*W = 128*128)
C = 64             # channels
HGRID = 128
SR = 8
NKV = 256          # (128/8)^2
EPS = 1e-5
N_CHUNK = 512      # query tokens per attention chunk
N_CHUNKS = N // N_CHUNK  # 32
TOK_TILE = 128
N_TILES = N // TOK_TILE  # 128
LOAD_BLK = 2048    # tokens per input DMA
N_LOADS = N // LOAD_BLK  # 8
TPB = LOAD_BLK // TOK_TILE  # token-tiles per load block (16)


def r32(ap):
    return ap.bitcast(F32R)


def build_graph():
    nc = bacc.Bacc("TRN2", target_bir_lowering=False, debug=False,
                   num_devices=N_CORES)

    x_ext = nc.declare_dram_parameter("x", [N, C], F32, isOutput=False)
    w_ext = {}
    for name in ("Wq", "Wk", "Wv", "Wp"):
        w_ext[name] = nc.declare_dram_parameter(name, [C, C], F32, isOutput=False)
    wsr_ext = nc.declare_dram_parameter("Wsr", [SR, SR, C, C], F32, isOutput=False)
    b_ext = {}
    for name in ("bq", "bk", "bv", "bsr", "bp", "gamma", "beta"):
        b_ext[name] = nc.declare_dram_parameter(name, [C], F32, isOutput=False)
    out_ext = nc.declare_dram_parameter("out", [N, C], F32, isOutput=True)

    with tile.TileContext(nc) as tc:
        with tc.tile_pool(name="const", bufs=1) as const_pool, \
             tc.tile_pool(name="persist", bufs=1) as persist_pool, \
             tc.tile_pool(name="xload", bufs=4) as xload_pool, \
             tc.tile_pool(name="work", bufs=2) as work_pool:

            # ---------- constants / weights ----------
            identity = const_pool.tile([128, 128], F32)
            make_identity(nc, identity[:])

            w_sb = {}
            for name in ("Wq", "Wk", "Wv", "Wp"):
                t = const_pool.tile([C, C], F32, tag=f"w_{name}")
                nc.sync.dma_start(t[:], w_ext[name][:])
                w_sb[name] = t
            wsr_st = const_pool.tile([C, SR * SR, C], F32, tag="wsr_st")
            nc.sync.dma_start(wsr_st[:], wsr_ext[:].rearrange("di dj c f -> c (di dj) f"))
            wsr_sb = const_pool.tile([C, SR * SR, C], F32R, tag="wsr")
            nc.vector.tensor_copy(wsr_sb[:, 0:SR * SR // 2, :], wsr_st[:, 0:SR * SR // 2, :])
            nc.scalar.copy(wsr_sb[:, SR * SR // 2:, :], wsr_st[:, SR * SR // 2:, :])
            b_sb = {}
            for name in ("bq", "bk", "bv", "bsr", "bp", "gamma", "beta"):
                t = const_pool.tile([C, 1], F32, tag=f"b_{name}")
                nc.sync.dma_start(t[:], b_ext[name][:].rearrange("(c one) -> c one", one=1))
                b_sb[name] = t

            # ---------- load x and transpose to xT [64, 16384] ----------
            xT = persist_pool.tile([C, N], F32R, tag="xT")
            with tc.tile_pool(name="pre_psum", bufs=2, space="PSUM") as pre_ps:
                xload_tiles = []
                for blk in range(N_LOADS):
                    xl = xload_pool.tile([TOK_TILE, TPB, C], F32, tag="xl")
                    nc.sync.dma_start(
                        xl[:],
                        x_ext[blk * LOAD_BLK:(blk + 1) * LOAD_BLK, :]
                        .rearrange("(j p) c -> p j c", p=TOK_TILE))
                    xload_tiles.append(xl)

                for g in range(N_TILES // 4):
                    ps = pre_ps.tile([C, 4 * TOK_TILE], F32, tag="xTp")
                    for u in range(4):
                        ti = 4 * g + u
                        nc.tensor.transpose(
                            ps[:, u * TOK_TILE:(u + 1) * TOK_TILE],
                            xload_tiles[ti // TPB][:, ti % TPB, :],
                            identity[:])
                    dst = xT[:, g * 512:(g + 1) * 512]
                    if g % 2 == 0:
                        nc.vector.tensor_copy(dst, ps[:])
                    else:
                        nc.scalar.copy(dst, ps[:])

                # ---------- spatial-reduction conv ----------
                xT5 = xT[:].rearrange("c (hi a wi b) -> c hi a wi b",
                                      hi=16, a=SR, wi=16, b=SR)
                xs_ps = pre_ps.tile([C, NKV], F32, tag="misc")
                for di in range(SR):
                    for dj in range(SR):
                        tap = di * SR + dj
                        nc.tensor.matmul(
                            xs_ps[:],
                            wsr_sb[:, tap, :],
                            xT5[:, :, di, :, dj],
                            start=(tap == 0), stop=(tap == SR * SR - 1))
                xs = work_pool.tile([C, NKV], F32, tag="xs")
                nc.scalar.activation(xs[:], xs_ps[:], AF.Identity, bias=b_sb["bsr"][:])

                # ---------- layernorm over channels (stats via ones-matmul) ----------
                ones64 = const_pool.tile([C, 1], F32, tag="ones64")
                nc.gpsimd.memset(ones64[:], 1.0)
                onesr1 = const_pool.tile([1, C], F32, tag="onesr1")
                nc.gpsimd.memset(onesr1[:], 1.0)

                sq = work_pool.tile([C, NKV], F32, tag="sq")
                nc.vector.tensor_mul(sq[:], xs[:], xs[:])
                m1_ps = pre_ps.tile([1, NKV], F32, tag="misc")
                nc.tensor.matmul(m1_ps[:], ones64[:], xs[:],
                                 start=True, stop=True)
                m2_ps = pre_ps.tile([1, NKV], F32, tag="misc")
                nc.tensor.matmul(m2_ps[:], ones64[:], sq[:],
                                 start=True, stop=True)
                mu = work_pool.tile([1, NKV], F32, tag="st_mu")
                nc.vector.tensor_scalar_mul(mu[:], m1_ps[:], 1.0 / C)
                ex2 = work_pool.tile([1, NKV], F32, tag="st_ex2")
                nc.vector.tensor_scalar_mul(ex2[:], m2_ps[:], 1.0 / C)
                var = work_pool.tile([1, NKV], F32, tag="st_var")
                nc.vector.tensor_mul(var[:], mu[:], mu[:])
                nc.vector.tensor_sub(var[:], ex2[:], var[:])
                eps_t = const_pool.tile([1, 1], F32, tag="eps")
                nc.gpsimd.memset(eps_t[:], EPS)
                sd = work_pool.tile([1, NKV], F32, tag="st_sd")
                nc.scalar.activation(sd[:], var[:], AF.Sqrt, bias=eps_t[:])
                rstd = work_pool.tile([1, NKV], F32, tag="st_rstd")
                nc.vector.reciprocal(rstd[:], sd[:])
                nmr = work_pool.tile([1, NKV], F32, tag="st_nmr")
                nc.vector.tensor_mul(nmr[:], mu[:], rstd[:])
                nc.vector.tensor_scalar_mul(nmr[:], nmr[:], -1.0)

                ab_ps = pre_ps.tile([C, 2 * NKV], F32, tag="misc")
                nc.tensor.matmul(ab_ps[:, 0:NKV], onesr1[:], rstd[:],
                                 start=True, stop=True)
                nc.tensor.matmul(ab_ps[:, NKV:2 * NKV], onesr1[:], nmr[:],
                                 start=True, stop=True)
                xsn = work_pool.tile([C, NKV], F32, tag="xsn")
                nc.vector.tensor_mul(xsn[:], xs[:], ab_ps[:, 0:NKV])
                nc.vector.tensor_add(xsn[:], xsn[:], ab_ps[:, NKV:2 * NKV])

                # ---------- K path: kq = Wq^T @ (gamma-folded K^T), bqk via bq ----------
                wkg = work_pool.tile([C, C], F32, tag="wf_k")
                nc.vector.tensor_scalar_mul(wkg[:], w_sb["Wk"][:], b_sb["gamma"][:])
                wvg = work_pool.tile([C, C], F32, tag="wf_v")
                nc.vector.tensor_scalar_mul(wvg[:], w_sb["Wv"][:], b_sb["gamma"][:])

                kT_ps = pre_ps.tile([C, NKV], F32, tag="misc")
                nc.tensor.matmul(kT_ps[:], wkg[:], xsn[:],
                                 start=True, stop=True)
                bkf_ps = pre_ps.tile([C, 1], F32, tag="vec")
                nc.tensor.matmul(bkf_ps[:], w_sb["Wk"][:], b_sb["beta"][:],
                                 start=True, stop=True)
                bkf = work_pool.tile([C, 1], F32, tag="bv_k")
                nc.vector.tensor_add(bkf[:], bkf_ps[:], b_sb["bk"][:])
                kT = work_pool.tile([C, NKV], F32, tag="kT")
                nc.scalar.activation(kT[:], kT_ps[:], AF.Identity, bias=bkf[:])

                wqT_ps = pre_ps.tile([C, C], F32, tag="vec")
                nc.tensor.transpose(wqT_ps[:], w_sb["Wq"][:], identity[0:C, 0:C])
                wqT = work_pool.tile([C, C], F32, tag="wf_q")
                nc.vector.tensor_copy(wqT[:], wqT_ps[:])
                kq_ps = pre_ps.tile([C, NKV], F32, tag="misc")
                nc.tensor.matmul(kq_ps[:], wqT[:], kT[:],
                                 start=True, stop=True)
                kq = persist_pool.tile([C, NKV], F32R, tag="kq")
                nc.vector.tensor_copy(kq[:], kq_ps[:])

                d_h = []
                for h in range(2):
                    bqk_ps = pre_ps.tile([TOK_TILE, 1], F32, tag="vec")
                    nc.tensor.matmul(bqk_ps[:],
                                     kT[:, h * 128:(h + 1) * 128],
                                     b_sb["bq"][:], start=True, stop=True)
                    dh = work_pool.tile([TOK_TILE, 1], F32, tag="dh")
                    nc.scalar.activation(dh[:], bqk_ps[:], AF.Exp, scale=0.125)
                    d_h.append(dh)

                # ---------- V path: vp = (xsn @ Wvg + bvf) @ Wp + bp ----------
                vT_ps = pre_ps.tile([C, NKV], F32, tag="misc")
                nc.tensor.matmul(vT_ps[:], wvg[:], xsn[:],
                                 start=True, stop=True)
                bvf_ps = pre_ps.tile([C, 1], F32, tag="vec")
                nc.tensor.matmul(bvf_ps[:], w_sb["Wv"][:], b_sb["beta"][:],
                                 start=True, stop=True)
                bvf = work_pool.tile([C, 1], F32, tag="bv_v")
                nc.vector.tensor_add(bvf[:], bvf_ps[:], b_sb["bv"][:])
                vT = work_pool.tile([C, NKV], F32, tag="vT_a")
                nc.scalar.activation(vT[:], vT_ps[:], AF.Identity, bias=bvf[:])

                vpT_ps = pre_ps.tile([C, NKV], F32, tag="misc")
                nc.tensor.matmul(vpT_ps[:], w_sb["Wp"][:], vT[:],
                                 start=True, stop=True)
                bvp_ps = pre_ps.tile([C, 1], F32, tag="vec")
                nc.tensor.matmul(bvp_ps[:], w_sb["Wp"][:], bvf[:],
                                 start=True, stop=True)
                bvp = work_pool.tile([C, 1], F32, tag="bv_p")
                nc.vector.tensor_add(bvp[:], bvp_ps[:], b_sb["bp"][:])
                vpT = work_pool.tile([C, NKV], F32, tag="vT_b")
                nc.scalar.activation(vpT[:], vpT_ps[:], AF.Identity, bias=bvp[:])

                # vp_aug halves in token layout, bf16, scaled by d_h; col 64 = d_h
                vps = []
                for h in range(2):
                    vpt_ps = pre_ps.tile([TOK_TILE, C], F32, tag="vec")
                    nc.tensor.transpose(vpt_ps[:], vpT[:, h * 128:(h + 1) * 128],
                                        identity[0:C, 0:C])
                    va = persist_pool.tile([TOK_TILE, C + 1], BF16, tag=f"vps{h}")
                    nc.vector.tensor_scalar_mul(va[:, 0:C], vpt_ps[:], d_h[h][:])
                    nc.vector.tensor_copy(va[:, C:C + 1], d_h[h][:])
                    vps.append(va)

            # ---------- attention ----------
            with tc.tile_pool(name="attn_psum", bufs=2, space="PSUM") as att_ps:
                for ci in range(N_CHUNKS):
                    s_ps = att_ps.tile([TOK_TILE, 2 * N_CHUNK], F32, tag="S")
                    xs_chunk = xT[:, ci * N_CHUNK:(ci + 1) * N_CHUNK]
                    nc.tensor.matmul(s_ps[:, 0:N_CHUNK], kq[:, 0:128],
                                     xs_chunk, start=True, stop=True)
                    nc.tensor.matmul(s_ps[:, N_CHUNK:2 * N_CHUNK], kq[:, 128:256],
                                     xs_chunk, start=True, stop=True)
                    e_t = work_pool.tile([TOK_TILE, 2 * N_CHUNK], BF16, tag="E")
                    nc.scalar.activation(e_t[:], s_ps[:], AF.Exp, scale=0.125)

                    y_ps = att_ps.tile([TOK_TILE, 4 * (C + 1)], F32, tag="Y")
                    for t in range(4):
                        ysl = y_ps[:, t * (C + 1):(t + 1) * (C + 1)]
                        nc.tensor.matmul(ysl, e_t[:, t * 128:(t + 1) * 128],
                                         vps[0][:], start=True, stop=False)
                        nc.tensor.matmul(ysl, e_t[:, N_CHUNK + t * 128:N_CHUNK + (t + 1) * 128],
                                         vps[1][:], start=False, stop=True)

                    yv = y_ps[:].rearrange("p (a b) -> p a b", a=4, b=C + 1)
                    r_t = work_pool.tile([TOK_TILE, 4, 1], F32, tag="r")
                    nc.vector.reciprocal(r_t[:], yv[:, :, C:C + 1])
                    y_t = work_pool.tile([TOK_TILE, 4, C], F32, tag="y")
                    nc.vector.tensor_mul(y_t[:], yv[:, :, 0:C],
                                         r_t[:].broadcast_to([TOK_TILE, 4, C]))
                    nc.sync.dma_start(
                        out_ext[ci * N_CHUNK:(ci + 1) * N_CHUNK, :]
                        .rearrange("(t p) f -> p t f", p=TOK_TILE),
                        y_t[:])

    nc.finalize()
    return nc


_NC_CACHE = None


def _get_nc():
    global _NC_CACHE
    if _NC_CACHE is None:
        _NC_CACHE = build_graph()
    return _NC_CACHE


def _make_in_maps(inputs):
    x = np.ascontiguousarray(np.asarray(inputs["x"], dtype=np.float32))
    B = x.shape[0]
    assert x.shape == (B, N, C) and B == N_CORES, x.shape
    common = {}
    for name in ("Wq", "Wk", "Wv", "Wp", "Wsr", "bq", "bk", "bv", "bsr",
                 "bp", "gamma", "beta"):
        common[name] = np.ascontiguousarray(np.asarray(inputs[name], dtype=np.float32))
    return [dict(common, x=np.ascontiguousarray(x[i])) for i in range(N_CORES)]


def run(inputs, trace=False):
    nc = _get_nc()
    in_maps = _make_in_maps(inputs)
    res = run_bass_kernel_spmd(nc, in_maps, list(range(N_CORES)), trace=trace)
    out = np.stack([np.asarray(res.results[i]["out"]) for i in range(N_CORES)])
    return out.astype(np.float32), res


def kernel(**inputs):
    out, _ = run(inputs, trace=False)
    return out


# revision 36
# speedup vs baseline: 1.5410x; 1.5410x over previous
"""Trainium2 Bass kernel for PVT-style spatial-reduction attention.

Problem (per batch element b of 8, one NeuronCore each — pure data parallel):
  q  = x @ Wq + bq                                  [16384, 64]
  xs = conv8x8s8(x.reshape(128,128,64), Wsr) + bsr  [256, 64]
  xs = LayerNorm(xs) * gamma + beta
  k  = xs @ Wk + bk ; v = xs @ Wv + bv              [256, 64]
  A  = softmax(q @ k.T / 8) ; o = A @ v             [16384, 64]
  out = o @ Wp + bp

Algebraic restructuring used on-device (all layouts channel-on-partition):
  - No Q tensor at all:  S^T[m, n] = sum_c kq[c, m] xT[c, n]
      with kq = Wq^T @ k^T  (i.e. Q-projection folded into K).
  - bq handled via d[m] = exp((k @ bq) / 8), folded diagonally into V:
      softmax(S + bqk 1^T) @ V == softmax-unnorm(S) @ diag(d) V / rowsum.
  - gamma/beta folded into Wk/Wv (+ bias terms), Wp+bp folded into V:
      vp = (xs_n @ Wvg + bvf) @ Wp + bp ; vp_aug = diag(d) [vp | 1].
  - Attention output computed directly in token layout with E = exp(S^T/8)
    as the matmul stationary operand:  y_un[n, :] = E^T-slices.T @ vp_aug,
    last column = softmax denominator; one reciprocal+multiply finishes.

Layout note: x^T lives as xT2 [128, 8192] — token-tile PAIRS transposed
[128,128] at a time on the PE; partitions 0:64 hold channels of even
token-tiles, 64:128 of odd tiles (image rows alternate parity, so each
conv tap reads one fixed half). S/conv contract at partition offset 0 or
64 with kq/Wsr duplicated on both halves.
"""

import os
import sys

import numpy as np

for _p in ("/root/.axon_site", "/root/.axon_site/_ro/trn_rl_repo",
           "/root/.axon_site/_ro/pypackages", "/opt/trn_rl_repo"):
    if os.path.isdir(_p) and _p not in sys.path:
        sys.path.append(_p)

import concourse.bass as bass  # noqa: E402
import concourse.mybir as mybir  # noqa: E402
import concourse.tile as tile  # noqa: E402
from concourse import bacc  # noqa: E402
from concourse.bass_utils import run_bass_kernel_spmd  # noqa: E402
from concourse.masks import make_identity  # noqa: E402

F32 = mybir.dt.float32
F32R = mybir.dt.float32r
BF16 = mybir.dt.bfloat16
AF = mybir.ActivationFunctionType

N_CORES = 8
N = 16384          # tokens per core (H*W = 128*128)
C = 64             # channels
SR = 8
NKV = 256          # (128/8)^2
EPS = 1e-5
N_CHUNK = 512      # query tokens per attention chunk
N_CHUNKS = N // N_CHUNK  # 32
TOK_TILE = 128
N_PAIRS = N // 256  # 64 pairs of token tiles
LOAD_BLK = 2048    # tokens per input DMA
N_LOADS = N // LOAD_BLK  # 8


def build_graph():
    nc = bacc.Bacc("TRN2", target_bir_lowering=False, debug=False,
                   num_devices=N_CORES)

    x_ext = nc.declare_dram_parameter("x", [N, C], F32, isOutput=False)
    w_ext = {}
    for name in ("Wq", "Wk", "Wv", "Wp"):
        w_ext[name] = nc.declare_dram_parameter(name, [C, C], F32, isOutput=False)
    wsr_ext = nc.declare_dram_parameter("Wsr", [SR, SR, C, C], F32, isOutput=False)
    b_ext = {}
    for name in ("bq", "bk", "bv", "bsr", "bp", "gamma", "beta"):
        b_ext[name] = nc.declare_dram_parameter(name, [C], F32, isOutput=False)
    out_ext = nc.declare_dram_parameter("out", [N, C], F32, isOutput=True)

    with tile.TileContext(nc) as tc:
        with tc.tile_pool(name="const", bufs=1) as const_pool, \
             tc.tile_pool(name="persist", bufs=1) as persist_pool, \
             tc.tile_pool(name="xload", bufs=8) as xload_pool, \
             tc.tile_pool(name="work", bufs=2) as work_pool:

            # ---------- DMA order: small weights, wsr0, x0, x1, wsr1, rest ----------
            # x layout: partition (rp, wh) = row-parity x w-pair; free
            # (rpair, (j, c)): per-partition 512B-contiguous DMA descriptors.
            # Wsr: half0 = taps 0..63 on partitions 0:64; half1 = taps shifted
            # by 1 (odd-dj partner) on partitions 64:128 -> K=128 tap pairs.
            xload_tiles = []

            def load_x(blk):
                xl = xload_pool.tile([TOK_TILE, LOAD_BLK // TOK_TILE * C], F32,
                                     tag="xl", name=f"xl{blk}")
                nc.sync.dma_start(
                    xl[:],
                    x_ext[blk * LOAD_BLK:(blk + 1) * LOAD_BLK, :]
                    .rearrange("(p u) c -> p (u c)", p=TOK_TILE))
                xload_tiles.append(xl)

            load_x(0)
            load_x(1)
            load_x(2)
            load_x(3)
            wsr_st = const_pool.tile([128, SR * SR, C], F32, tag="wsr_st")
            wsr_dview = wsr_ext[:].rearrange("di dj c f -> c (di dj) f")
            nc.sync.dma_start(wsr_st[0:C], wsr_dview)
            load_x(4)
            load_x(5)
            load_x(6)
            load_x(7)
            # shifted dup half via on-chip SBUF->SBUF DMA (saves 1MB of HBM)
            nc.sync.dma_start(wsr_st[C:128, 0:SR * SR - 1, :],
                              wsr_st[0:C, 1:SR * SR, :])
            w_sb = {}
            for name in ("Wq", "Wk", "Wv", "Wp"):
                t = const_pool.tile([C, C], F32, tag=f"w_{name}")
                nc.sync.dma_start(t[:], w_ext[name][:])
                w_sb[name] = t
            b_sb = {}
            for name in ("bq", "bk", "bv", "bsr", "bp", "gamma", "beta"):
                t = const_pool.tile([C, 1], F32, tag=f"b_{name}")
                nc.sync.dma_start(t[:], b_ext[name][:].rearrange("(c one) -> c one", one=1))
                b_sb[name] = t

            identity = const_pool.tile([128, 128], F32)
            make_identity(nc, identity[:])
            wsr_sb = const_pool.tile([128, SR * SR, C], F32R, tag="wsr")
            nc.vector.tensor_copy(wsr_sb[:, 0:SR * SR // 2, :], wsr_st[:, 0:SR * SR // 2, :])
            nc.scalar.copy(wsr_sb[:, SR * SR // 2:, :], wsr_st[:, SR * SR // 2:, :])

            # ---------- pair-transpose x into xT2 [128, 8192] ----------
            # partition h*64+c: channel c of tile-parity h; free pp*128+t.
            xT2 = persist_pool.tile([128, N // 2], F32R, tag="xT2")
            with tc.tile_pool(name="pre_psum", bufs=2, space="PSUM") as pre_ps:
                xT2v = xT2[:].rearrange(
                    "p (b jp dh i1 di jh) -> p b jp dh i1 di jh",
                    b=8, jp=2, dh=4, i1=2, di=8, jh=8)
                xs_ps = pre_ps.tile([C, NKV], F32, tag="conv", bufs=1)

                def conv_half(bh):
                    # taps over blocks [4bh, 4bh+4) -> kv columns [128bh, 128bh+128)
                    for k, dj in enumerate(range(0, SR, 2)):
                        for di in range(SR):
                            tap = di * SR + dj
                            nc.tensor.matmul(
                                xs_ps[:, 128 * bh:128 * bh + 128],
                                wsr_sb[:, tap, :],
                                xT2v[:, 4 * bh:4 * bh + 4, :, dj // 2, :, di, :],
                                start=(k == 0 and di == 0),
                                stop=(k == SR // 2 - 1 and di == SR - 1))

                for g in range(N_PAIRS // 4):
                    ps = pre_ps.tile([128, 512], F32, tag="xTp")
                    for u in range(4):
                        pp = 4 * g + u
                        blk, pj = pp // 8, pp % 8
                        nc.tensor.transpose(
                            ps[:, u * TOK_TILE:(u + 1) * TOK_TILE],
                            xload_tiles[blk][:, pj * 128:(pj + 1) * 128],
                            identity[:])
                    nc.vector.tensor_copy(xT2[:, g * 512:(g + 1) * 512], ps[:])
                    if g == 7:
                        conv_half(0)
                conv_half(1)
                xs2 = work_pool.tile([C, 2 * NKV], F32R, tag="sq")
                xs = xs2[:, 0:NKV]
                nc.scalar.activation(xs, xs_ps[:], AF.Identity, bias=b_sb["bsr"][:])

                # ---------- weight-only precompute (parallel to conv/LN) ----------
                wkg = work_pool.tile([C, C], F32, tag="wf_k")
                nc.vector.tensor_scalar_mul(wkg[:], w_sb["Wk"][:], b_sb["gamma"][:])
                wvg = work_pool.tile([C, C], F32, tag="wf_v")
                nc.vector.tensor_scalar_mul(wvg[:], w_sb["Wv"][:], b_sb["gamma"][:])

                wqT_ps = pre_ps.tile([C, C], F32, tag="vec", bufs=2)
                nc.tensor.transpose(wqT_ps[:], w_sb["Wq"][:], identity[0:C, 0:C])
                wqT2 = work_pool.tile([C, 2 * C], F32, tag="wf_q")
                nc.vector.tensor_copy(wqT2[:, 0:C], wqT_ps[:])
                nc.vector.tensor_copy(wqT2[:, C:2 * C], wqT_ps[:])
                wkgT_ps = pre_ps.tile([C, C], F32, tag="vec", bufs=2)
                nc.tensor.transpose(wkgT_ps[:], wkg[:], identity[0:C, 0:C])
                wkgT = work_pool.tile([C, C], F32, tag="wf_kT")
                nc.vector.tensor_copy(wkgT[:], wkgT_ps[:])
                wvgT_ps = pre_ps.tile([C, C], F32, tag="vec", bufs=2)
                nc.tensor.transpose(wvgT_ps[:], wvg[:], identity[0:C, 0:C])
                wvgT = work_pool.tile([C, C], F32, tag="wf_vT")
                nc.vector.tensor_copy(wvgT[:], wvgT_ps[:])

                bkf_ps = pre_ps.tile([C, 1], F32, tag="vec", bufs=2)
                nc.tensor.matmul(bkf_ps[:], w_sb["Wk"][:], b_sb["beta"][:],
                                 start=True, stop=True)
                bkf = work_pool.tile([C, 1], F32, tag="bv_k")
                nc.vector.tensor_add(bkf[:], bkf_ps[:], b_sb["bk"][:])
                bvf_ps = pre_ps.tile([C, 1], F32, tag="vec", bufs=2)
                nc.tensor.matmul(bvf_ps[:], w_sb["Wv"][:], b_sb["beta"][:],
                                 start=True, stop=True)
                bvf = work_pool.tile([C, 1], F32, tag="bv_v")
                nc.vector.tensor_add(bvf[:], bvf_ps[:], b_sb["bv"][:])

                mxT2_ps = pre_ps.tile([C, 2 * C], F32, tag="vec", bufs=2)
                nc.tensor.matmul(mxT2_ps[:], wkgT[:], wqT2[:], start=True, stop=True)
                mxT2 = work_pool.tile([C, 2 * C], F32, tag="wf_mx")
                nc.vector.tensor_copy(mxT2[:], mxT2_ps[:])
                cvec2_ps = pre_ps.tile([2 * C, 1], F32, tag="vec", bufs=2)
                nc.tensor.matmul(cvec2_ps[:], wqT2[:], bkf[:], start=True, stop=True)
                cvec2 = work_pool.tile([2 * C, 1], F32, tag="bv_c")
                nc.vector.tensor_copy(cvec2[:], cvec2_ps[:])
                wb_ps = pre_ps.tile([C, 1], F32, tag="vec", bufs=2)
                nc.tensor.matmul(wb_ps[:], wkgT[:], b_sb["bq"][:], start=True, stop=True)
                wb = work_pool.tile([C, 1], F32, tag="bv_wb")
                nc.vector.tensor_copy(wb[:], wb_ps[:])
                nx_ps = pre_ps.tile([C, C], F32, tag="vec", bufs=2)
                nc.tensor.matmul(nx_ps[:], wvgT[:], w_sb["Wp"][:], start=True, stop=True)
                nx = work_pool.tile([C, C], F32, tag="wf_nx")
                nc.vector.tensor_copy(nx[:], nx_ps[:])
                bvp_ps = pre_ps.tile([C, 1], F32, tag="vec", bufs=2)
                nc.tensor.matmul(bvp_ps[:], w_sb["Wp"][:], bvf[:], start=True, stop=True)
                bvp = work_pool.tile([C, 1], F32, tag="bv_p")
                nc.vector.tensor_add(bvp[:], bvp_ps[:], b_sb["bp"][:])

                # ---------- layernorm over channels (stats via ones-matmul) ----------
                ones_st = const_pool.tile([C, 1], F32, tag="ones_st")
                nc.gpsimd.memset(ones_st[:], 1.0)
                onesr_st = const_pool.tile([1, C], F32, tag="onesr_st")
                nc.gpsimd.memset(onesr_st[:], 1.0)
                ones64 = const_pool.tile([C, 1], F32R, tag="ones64")
                nc.vector.tensor_copy(ones64[:], ones_st[:])
                onesr1 = const_pool.tile([1, C], F32R, tag="onesr1")
                nc.vector.tensor_copy(onesr1[:], onesr_st[:])

                # xs2 = [xs | xs^2] so one ones-matmul yields [m1 | m2]
                nc.vector.tensor_mul(xs2[:, NKV:2 * NKV], xs, xs)
                m12_ps = pre_ps.tile([1, 2 * NKV], F32, tag="misc", bufs=1)
                nc.tensor.matmul(m12_ps[:], ones64[:], xs2[:], start=True, stop=True)
                mu = work_pool.tile([1, NKV], F32, tag="st_mu")
                nc.vector.tensor_scalar_mul(mu[:], m12_ps[:, 0:NKV], 1.0 / C)
                mu2 = work_pool.tile([1, NKV], F32, tag="st_ex2")
                nc.vector.tensor_mul(mu2[:], mu[:], mu[:])
                var = work_pool.tile([1, NKV], F32, tag="st_var")
                nc.vector.scalar_tensor_tensor(
                    var[:], m12_ps[:, NKV:2 * NKV], 1.0 / C, mu2[:],
                    op0=mybir.AluOpType.mult, op1=mybir.AluOpType.subtract)
                eps_t = const_pool.tile([1, 1], F32, tag="eps")
                nc.gpsimd.memset(eps_t[:], EPS)
                # ab = [rstd | -mu*rstd] contiguous so one K=1 matmul broadcasts
                ab = work_pool.tile([1, 2 * NKV], F32R, tag="st_rstd")
                nc.scalar.activation(ab[:, 0:NKV], var[:], AF.Abs_reciprocal_sqrt,
                                     bias=eps_t[:])
                nc.vector.scalar_tensor_tensor(
                    ab[:, NKV:2 * NKV], mu[:], -1.0, ab[:, 0:NKV],
                    op0=mybir.AluOpType.mult, op1=mybir.AluOpType.mult)
                ab_ps = pre_ps.tile([C, 2 * NKV], F32, tag="misc", bufs=1)
                nc.tensor.matmul(ab_ps[:], onesr1[:], ab[:], start=True, stop=True)
                xsn = work_pool.tile([C, NKV], F32, tag="xsn")
                nc.vector.tensor_mul(xsn[:], xs, ab_ps[:, 0:NKV])
                nc.vector.tensor_add(xsn[:], xsn[:], ab_ps[:, NKV:2 * NKV])

                # ---------- K/V via precomputed weight products ----------
                # kq2 = MxT2^T @ xsn + cvec2;  MxT = Wkg^T-matmul of WqT
                # vpT = Nx^T @ xsn + bvp;      Nx = Wvg @ Wp
                # bqk = xsn^T @ wb (wb = Wkg^T-fold of bq); const term cancels
                # in softmax so it is dropped.
                kq_ps = pre_ps.tile([128, NKV], F32, tag="misc", bufs=1)
                nc.tensor.matmul(kq_ps[:], mxT2[:], xsn[:], start=True, stop=True)
                kq2 = persist_pool.tile([128, NKV], F32R, tag="kq2")
                nc.scalar.activation(kq2[:], kq_ps[:], AF.Identity, bias=cvec2[:])

                d_h = []
                for h in range(2):
                    bqk_ps = pre_ps.tile([TOK_TILE, 1], F32, tag="vec", bufs=2)
                    nc.tensor.matmul(bqk_ps[:],
                                     xsn[:, h * 128:(h + 1) * 128],
                                     wb[:], start=True, stop=True)
                    dh = work_pool.tile([TOK_TILE, 1], F32, tag="dh")
                    nc.scalar.activation(dh[:], bqk_ps[:], AF.Exp, scale=0.125)
                    d_h.append(dh)

                vpT_ps = pre_ps.tile([C, NKV], F32, tag="misc", bufs=1)
                nc.tensor.matmul(vpT_ps[:], nx[:], xsn[:], start=True, stop=True)
                vpT = work_pool.tile([C, NKV], F32, tag="vT_b")
                nc.scalar.activation(vpT[:], vpT_ps[:], AF.Identity, bias=bvp[:])

                # vp_aug halves in token layout, bf16, scaled by d_h; col 64 = d_h
                vps = []
                for h in range(2):
                    vpt_ps = pre_ps.tile([TOK_TILE, C], F32, tag="vec", bufs=2)
                    nc.tensor.transpose(vpt_ps[:], vpT[:, h * 128:(h + 1) * 128],
                                        identity[0:C, 0:C])
                    va = persist_pool.tile([TOK_TILE, C + 1], BF16, tag=f"vps{h}")
                    nc.vector.tensor_scalar_mul(va[:, 0:C], vpt_ps[:], d_h[h][:])
                    nc.vector.tensor_copy(va[:, C:C + 1], d_h[h][:])
                    vps.append(va)

            # ---------- attention ----------
            # chunk ci = tiles 4ci..4ci+3; xT2 free block [256ci, 256ci+256)
            # holds even tiles (4ci, 4ci+2) on partitions 0:64 and odd tiles
            # (4ci+1, 4ci+3) on 64:128. E col-block order per m-half:
            # [tile+0 | tile+2 | tile+1 | tile+3].
            with tc.tile_pool(name="attn_psum_s", bufs=3, space="PSUM") as att_ps_s, \
                 tc.tile_pool(name="attn_psum_y", bufs=2, space="PSUM") as att_ps_y:
                for ci in range(N_CHUNKS):
                    s_ps = att_ps_s.tile([TOK_TILE, 2 * N_CHUNK], F32, tag="S")
                    xb = xT2[:, 256 * ci:256 * (ci + 1)]
                    for par in range(2):   # bank `par`: tokens of parity par
                        o = C * par
                        for mh in range(2):
                            base = par * N_CHUNK + mh * 256
                            nc.tensor.matmul(s_ps[:, base:base + 256],
                                             kq2[o:o + C, mh * 128:(mh + 1) * 128],
                                             xb[o:o + C, :], start=True, stop=True)
                    e_t = work_pool.tile([TOK_TILE, 2 * N_CHUNK], BF16, tag="E", bufs=3)
                    nc.scalar.activation(e_t[:], s_ps[:], AF.Exp, scale=0.125)

                    y_ps = att_ps_y.tile([TOK_TILE, 4 * (C + 1)], F32, tag="Y")
                    for u in range(4):
                        ysl = y_ps[:, u * (C + 1):(u + 1) * (C + 1)]
                        b, j = u // 2, u % 2
                        col0 = 512 * j + 128 * b
                        nc.tensor.matmul(ysl, e_t[:, col0:col0 + 128],
                                         vps[0][:], start=True, stop=False)
                        nc.tensor.matmul(ysl, e_t[:, 256 + col0:256 + col0 + 128],
                                         vps[1][:], start=False, stop=True)

                    yv = y_ps[:].rearrange("p (a b) -> p a b", a=4, b=C + 1)
                    r_t = work_pool.tile([TOK_TILE, 4, 1], F32, tag="r", bufs=3)
                    nc.vector.reciprocal(r_t[:], yv[:, :, C:C + 1])
                    y_t = work_pool.tile([TOK_TILE, 4, C], F32, tag="y", bufs=3)
                    nc.vector.tensor_mul(y_t[:], yv[:, :, 0:C],
                                         r_t[:].broadcast_to([TOK_TILE, 4, C]))
                    ov = out_ext[:].rearrange("(b p ur j) f -> b p ur j f",
                                              b=8, p=TOK_TILE, ur=8, j=2)
                    nc.sync.dma_start(
                        ov[ci // 4, :, 2 * (ci % 4):2 * (ci % 4) + 2, :, :],
                        y_t[:].rearrange("p (s j) f -> p s j f", s=2, j=2))

    nc.finalize()
    return nc


_NC_CACHE = None


def _get_nc():
    global _NC_CACHE
    if _NC_CACHE is None:
        _NC_CACHE = build_graph()
    return _NC_CACHE


def _make_in_maps(inputs):
    x = np.ascontiguousarray(np.asarray(inputs["x"], dtype=np.float32))
    B = x.shape[0]
    assert x.shape == (B, N, C) and B == N_CORES, x.shape
    common = {}
    for name in ("Wq", "Wk", "Wv", "Wp", "Wsr", "bq", "bk", "bv", "bsr",
                 "bp", "gamma", "beta"):
        common[name] = np.ascontiguousarray(np.asarray(inputs[name], dtype=np.float32))
    return [dict(common, x=np.ascontiguousarray(x[i])) for i in range(N_CORES)]


def run(inputs, trace=False):
    nc = _get_nc()
    in_maps = _make_in_maps(inputs)
    res = run_bass_kernel_spmd(nc, in_maps, list(range(N_CORES)), trace=trace)
    out = np.stack([np.asarray(res.results[i]["out"]) for i in range(N_CORES)])
    return out.astype(np.float32), res


def kernel(**inputs):
    out, _ = run(inputs, trace=False)
    return out
